# revision 2
# baseline (speedup 1.0000x reference)
"""C51 categorical-DQN histogram projection on Trainium2, 8-core data-parallel.

v2: no DRAM round-trip, no indirect DMA. Per-row window extraction from the
in-SBUF padded prefix table via a 5-pass in-place predicated shift cascade
(binary decomposition of the per-row shift t = 12 - s, t in [0, 26]).
Tables are carry-free fp16, built by one LPC-form scan (state = mask*state + p)
that also regenerates the 0 / total pads every tile. Tent MAC in fp16.

Row mapping is p-major: within a tile of 4096 rows, partition p owns rows
[p*32, (p+1)*32), so all HBM transfers are contiguous 128B+ runs per partition.
"""
import sys
sys.path.insert(0, "/opt/trn_rl_repo")
import numpy as np
from concourse import bass, bacc, mybir, tile
from concourse.bass_utils import run_bass_kernel_spmd

F32 = mybir.dt.float32
F16 = mybir.dt.float16
I32 = mybir.dt.int32
OP = mybir.AluOpType
AF = mybir.ActivationFunctionType

P = 128
A = 51
B_TOTAL = 1048576
N_CORES = 8
BC = B_TOTAL // N_CORES
GAMMA = 0.99
G = 32
TILE = P * G
SP = 80           # padded table width per row (fp16): 13 pad0 | 52 table | 15 pad1
TBL = 13          # column of T[0] within a segment
SMIN, SMAX = -14, 12   # si clamp; t = 12 - si in [0, 26]
PACKED = True     # run the 16/8/4/2 cascade passes on int32-viewed fp16 pairs


def _host_consts():
    j001n = (-0.01 * np.arange(54, dtype=np.float32))[None, :].repeat(P, 0)
    return j001n


def _build_nc(Bc):
    T = Bc // TILE
    FA = G * A
    FS = G * SP

    nc = bacc.Bacc("TRN2", target_bir_lowering=False, debug=False)
    pr = nc.dram_tensor("pdist", [Bc, A], F32, kind="ExternalInput")
    rr = nc.dram_tensor("reward", [Bc], F32, kind="ExternalInput")
    mm = nc.dram_tensor("mask", [Bc], I32, kind="ExternalInput")
    j001n_c = nc.dram_tensor("j001n", [P, 54], F32, kind="ExternalInput")
    mo = nc.dram_tensor("mout", [Bc, A], F32, kind="ExternalOutput")

    prf = pr[:, :].rearrange("b a -> (b a)")
    mof = mo[:, :].rearrange("b a -> (b a)")

    with tile.TileContext(nc) as tc:
        with tc.tile_pool(name="const", bufs=1) as cpool:
            j001n = cpool.tile([P, 54], F32)
            nc.sync.dma_start(out=j001n[:], in_=j001n_c[:, :])
            biases = []
            for k in range(3):
                bk = cpool.tile([P, 1], F32, tag=f"bias{k}")
                nc.vector.memset(bk[:], float(k))
                biases.append(bk)
            bone = cpool.tile([P, 1], F32, tag="bone")
            nc.vector.memset(bone[:], 1.0)
            zc16 = cpool.tile([P, 1], F16, tag="zc16")
            nc.vector.memset(zc16[:], 0.0)
            # LPC scan reset mask: 1 everywhere, 0 at segment col 0
            d0m = cpool.tile([P, FS], F16, tag="d0m")
            nc.vector.memset(d0m[:], 1.0)
            nc.vector.memset(
                bass.AP(d0m[:].tensor, d0m[:].offset, [d0m[:].ap[0], [SP, G]]), 0.0)
            # staged scan inputs (double-buffered manually; const cols stay 0)
            staged = []
            for pp in range(2):
                sg = cpool.tile([P, FS], F16, tag=f"staged{pp}")
                nc.vector.memset(sg[:], 0.0)
                staged.append(sg)

            with tc.tile_pool(name="sb", bufs=2) as pool:
                for t in range(T):
                    tbase = t * TILE
                    sg = staged[t % 2]

                    pt = pool.tile([P, FA], F32, tag="pt")
                    nc.sync.dma_start(
                        out=pt[:], in_=bass.AP(prf.tensor, tbase * A,
                                               [[G * A, P], [1, G * A]]))
                    rt = pool.tile([P, G], F32, tag="rt")
                    nc.sync.dma_start(
                        out=rt[:], in_=bass.AP(rr[:].tensor, tbase, [[G, P], [1, G]]))
                    mkt = pool.tile([P, G], I32, tag="mkt")
                    nc.sync.dma_start(
                        out=mkt[:], in_=bass.AP(mm[:].tensor, tbase, [[G, P], [1, G]]))

                    def ptv(off, *dims):
                        h = pt[:]
                        return bass.AP(h.tensor, h.offset + off, [h.ap[0]] + list(dims))

                    # stage p into table slots: staged cols 14..64 <- p[0..50]
                    sgh = sg[:]

                    def sgv(off, *dims):
                        return bass.AP(sgh.tensor, sgh.offset + off,
                                       [sgh.ap[0]] + list(dims))
                    nc.vector.tensor_copy(
                        out=sgv(TBL + 1, [SP, G], [1, A]), in_=ptv(0, [A, G], [1, A]))

                    # LPC scan -> carry-free fp16 padded tables (pads included)
                    X = pool.tile([P, FS], F16, tag="X")
                    nc.vector.tensor_tensor_scan(
                        out=X[:], data0=d0m[:], data1=sg[:], initial=0.0,
                        op0=OP.mult, op1=OP.add)
                    Xh = X[:]

                    def xv(off, *dims):
                        return bass.AP(Xh.tensor, Xh.offset + off,
                                       [Xh.ap[0]] + list(dims))

                    # per-row scalars (baseline math, SMIN/SMAX tightened)
                    notmk = pool.tile([P, G], I32, tag="notmk")
                    nc.vector.tensor_scalar(
                        out=notmk[:], in0=mkt[:], scalar1=1, scalar2=None,
                        op0=OP.bitwise_xor)
                    mf = pool.tile([P, G], F32, tag="mf")
                    nc.vector.tensor_copy(out=mf[:], in_=mkt[:])
                    a1 = pool.tile([P, G], F32, tag="a1")
                    nc.vector.tensor_scalar(out=a1[:], in0=rt[:], scalar1=2.5,
                                            scalar2=0.25, op0=OP.mult, op1=OP.add)
                    qt = pool.tile([P, G], F32, tag="qt")
                    nc.vector.tensor_scalar(out=qt[:], in0=rt[:], scalar1=2.5,
                                            scalar2=25.0, op0=OP.mult, op1=OP.add)
                    nc.vector.tensor_scalar(out=qt[:], in0=qt[:], scalar1=0.0,
                                            scalar2=50.0, op0=OP.max, op1=OP.min)
                    nc.vector.tensor_scalar(out=qt[:], in0=qt[:],
                                            scalar1=GAMMA * 25.0, scalar2=None,
                                            op0=OP.subtract)
                    al = pool.tile([P, G], F32, tag="al")
                    nc.vector.tensor_tensor(out=al[:], in0=a1[:], in1=qt[:],
                                            op=OP.subtract)
                    nc.vector.tensor_tensor(out=al[:], in0=al[:], in1=mf[:],
                                            op=OP.mult)
                    nc.vector.tensor_tensor(out=al[:], in0=al[:], in1=qt[:],
                                            op=OP.add)
                    sf = pool.tile([P, G], F32, tag="sf")
                    nc.vector.tensor_scalar(out=sf[:], in0=al[:], scalar1=-0.5,
                                            scalar2=None, op0=OP.add)
                    si = pool.tile([P, G], I32, tag="si")
                    nc.vector.tensor_copy(out=si[:], in_=sf[:])
                    nc.vector.tensor_scalar(out=si[:], in0=si[:], scalar1=SMIN,
                                            scalar2=SMAX, op0=OP.max, op1=OP.min)
                    nc.vector.tensor_copy(out=sf[:], in_=si[:])
                    rho = pool.tile([P, G], F32, tag="rho")
                    nc.vector.tensor_scalar(out=rho[:], in0=sf[:], scalar1=-GAMMA,
                                            scalar2=-GAMMA, op0=OP.mult, op1=OP.add)
                    nc.vector.tensor_tensor(out=rho[:], in0=rho[:], in1=al[:],
                                            op=OP.add)
                    ti = pool.tile([P, G], I32, tag="ti")
                    nc.vector.tensor_scalar(out=ti[:], in0=si[:], scalar1=-1,
                                            scalar2=12, op0=OP.mult, op1=OP.add)

                    # mask=0 rows: table -> step at virtual atom 25
                    def nmk(n):
                        h = notmk[:]
                        return bass.AP(h.tensor, h.offset, [h.ap[0], [1, G], [0, n]])
                    nc.vector.copy_predicated(
                        out=xv(TBL + 1, [SP, G], [1, 25]), mask=nmk(25),
                        data=bass.AP(zc16[:].tensor, zc16[:].offset,
                                     [zc16[:].ap[0], [0, G], [0, 25]]))
                    nc.vector.copy_predicated(
                        out=xv(TBL + 26, [SP, G], [1, 26]), mask=nmk(26),
                        data=xv(TBL + 51, [SP, G], [0, 26]))

                    # shift cascade: X[seg, i] <- X[seg, i + t] via bits of t
                    bitm = pool.tile([P, G], I32, tag="bitm")
                    if PACKED:
                        X32 = Xh.bitcast(I32)

                        def x32v(off, *dims):
                            return bass.AP(X32.tensor, X32.offset + off,
                                           [X32.ap[0]] + list(dims))
                        for b, w in ((16, 64), (8, 61), (4, 57), (2, 55)):
                            nc.vector.tensor_scalar(
                                out=bitm[:], in0=ti[:], scalar1=b, scalar2=None,
                                op0=OP.bitwise_and)
                            wp = (w + 1) // 2
                            nc.vector.copy_predicated(
                                out=x32v(0, [SP // 2, G], [1, wp]),
                                mask=bass.AP(bitm[:].tensor, bitm[:].offset,
                                             [bitm[:].ap[0], [1, G], [0, wp]]),
                                data=x32v(b // 2, [SP // 2, G], [1, wp]))
                    else:
                        for b, w in ((16, 64), (8, 61), (4, 57), (2, 55)):
                            nc.vector.tensor_scalar(
                                out=bitm[:], in0=ti[:], scalar1=b, scalar2=None,
                                op0=OP.bitwise_and)
                            nc.vector.copy_predicated(
                                out=xv(0, [SP, G], [1, w]),
                                mask=bass.AP(bitm[:].tensor, bitm[:].offset,
                                             [bitm[:].ap[0], [1, G], [0, w]]),
                                data=xv(b, [SP, G], [1, w]))
                    nc.vector.tensor_scalar(
                        out=bitm[:], in0=ti[:], scalar1=1, scalar2=None,
                        op0=OP.bitwise_and)
                    nc.vector.copy_predicated(
                        out=xv(0, [SP, G], [1, 54]),
                        mask=bass.AP(bitm[:].tensor, bitm[:].offset,
                                     [bitm[:].ap[0], [1, G], [0, 54]]),
                        data=xv(1, [SP, G], [1, 54]))

                    # window diffs wd[i] = W[i+1] - W[i], fp16
                    wd = pool.tile([P, G * 54], F16, tag="wd")
                    wdh = wd[:]

                    def wdv(off, *dims):
                        return bass.AP(wdh.tensor, wdh.offset + off,
                                       [wdh.ap[0]] + list(dims))
                    nc.vector.tensor_tensor(
                        out=wdv(0, [54, G], [1, 53]), in0=xv(1, [SP, G], [1, 53]),
                        in1=xv(0, [SP, G], [1, 53]), op=OP.subtract)

                    # tent args Y = rho - 0.01*i
                    Y = pool.tile([P, G * 54], F32, tag="Y")
                    nc.vector.tensor_tensor(
                        out=Y[:],
                        in0=bass.AP(rho[:].tensor, rho[:].offset,
                                    [rho[:].ap[0], [1, G], [0, 54]]),
                        in1=bass.AP(j001n[:].tensor, j001n[:].offset,
                                    [j001n[:].ap[0], [0, G], [1, 54]]),
                        op=OP.add)
                    Yh = Y[:]

                    def yv(off, *dims):
                        return bass.AP(Yh.tensor, Yh.offset + off,
                                       [Yh.ap[0]] + list(dims))

                    # 3-tap tent MAC in fp16
                    mt_ = pool.tile([P, FA], F16, tag="mt_")
                    au = pool.tile([P, FA], F16, tag="au")
                    tmp = pool.tile([P, FA], F16, tag="tmp")
                    for k in range(3):
                        nc.scalar.activation(
                            out=au[:], in_=yv(k, [54, G], [1, A]),
                            func=AF.Abs, bias=biases[k][:], scale=1.0)
                        nc.scalar.activation(
                            out=au[:], in_=au[:], func=AF.Relu, bias=bone[:],
                            scale=-1.0)
                        if k == 0:
                            nc.vector.tensor_tensor(
                                out=mt_[:], in0=au[:], in1=wdv(0, [54, G], [1, A]),
                                op=OP.mult)
                        else:
                            nc.vector.tensor_tensor(
                                out=tmp[:], in0=au[:], in1=wdv(k, [54, G], [1, A]),
                                op=OP.mult)
                            nc.vector.tensor_tensor(
                                out=mt_[:], in0=mt_[:], in1=tmp[:], op=OP.add)

                    # edge corrections: d0 (bin 0), d5 (bin 50)
                    d0 = pool.tile([P, G], F32, tag="d0")
                    nc.vector.tensor_copy(out=d0[:], in_=xv(0, [SP, G]))
                    cx = pool.tile([P, G], F32, tag="cx")
                    t2 = pool.tile([P, G], F32, tag="t2")
                    for i in (0, 1):
                        nc.vector.tensor_scalar(out=cx[:], in0=rho[:], scalar1=-1.0,
                                                scalar2=-GAMMA * i, op0=OP.mult,
                                                op1=OP.add)
                        nc.vector.tensor_scalar(out=cx[:], in0=cx[:], scalar1=0.0,
                                                scalar2=1.0, op0=OP.max, op1=OP.min)
                        nc.vector.tensor_tensor(out=t2[:], in0=cx[:],
                                                in1=wdv(i, [54, G]), op=OP.mult)
                        nc.vector.tensor_tensor(out=d0[:], in0=d0[:], in1=t2[:],
                                                op=OP.add)
                    d5 = pool.tile([P, G], F32, tag="d5")
                    nc.vector.tensor_scalar(out=d5[:], in0=xv(53, [SP, G]),
                                            scalar1=-1.0, scalar2=1.0,
                                            op0=OP.mult, op1=OP.add)
                    for i in (50, 51, 52):
                        nc.vector.tensor_scalar(out=cx[:], in0=rho[:],
                                                scalar1=GAMMA * i - 50.0,
                                                scalar2=None, op0=OP.add)
                        nc.vector.tensor_scalar(out=cx[:], in0=cx[:], scalar1=0.0,
                                                scalar2=1.0, op0=OP.max, op1=OP.min)
                        nc.vector.tensor_tensor(out=t2[:], in0=cx[:],
                                                in1=wdv(i, [54, G]), op=OP.mult)
                        nc.vector.tensor_tensor(out=d5[:], in0=d5[:], in1=t2[:],
                                                op=OP.add)
                    mh = mt_[:]
                    nc.vector.tensor_tensor(
                        out=bass.AP(mh.tensor, mh.offset, [mh.ap[0], [A, G]]),
                        in0=bass.AP(mh.tensor, mh.offset, [mh.ap[0], [A, G]]),
                        in1=d0[:], op=OP.add)
                    nc.vector.tensor_tensor(
                        out=bass.AP(mh.tensor, mh.offset + 50, [mh.ap[0], [A, G]]),
                        in0=bass.AP(mh.tensor, mh.offset + 50, [mh.ap[0], [A, G]]),
                        in1=d5[:], op=OP.add)

                    mo_t = pool.tile([P, FA], F32, tag="mo_t")
                    nc.vector.tensor_copy(out=mo_t[:], in_=mt_[:])
                    nc.sync.dma_start(
                        out=bass.AP(mof.tensor, tbase * A, [[G * A, P], [1, G * A]]),
                        in_=mo_t[:])
    nc.compile()
    return nc


_NC_CACHE = {}


def kernel(batch_reward, max_next_dist, supports, non_final_mask):
    assert max_next_dist.shape == (B_TOTAL, A)
    if "nc" not in _NC_CACHE:
        _NC_CACHE["nc"] = _build_nc(BC)
    nc = _NC_CACHE["nc"]
    j001n = _host_consts()
    in_maps = []
    for c in range(N_CORES):
        s = slice(c * BC, (c + 1) * BC)
        in_maps.append({
            "pdist": np.ascontiguousarray(max_next_dist[s]).astype(np.float32),
            "reward": np.ascontiguousarray(batch_reward[s]).astype(np.float32),
            "mask": np.ascontiguousarray(non_final_mask[s]).astype(np.int32),
            "j001n": j001n,
        })
    res = run_bass_kernel_spmd(nc, in_maps, core_ids=list(range(N_CORES)))
    return np.concatenate([res.results[c]["mout"] for c in range(N_CORES)], axis=0)


# revision 20
# speedup vs baseline: 1.0210x; 1.0210x over previous
"""C51 categorical-DQN histogram projection on Trainium2, 8-core data-parallel.

v2: no DRAM round-trip, no indirect DMA. Per-row window extraction from the
in-SBUF padded prefix table via a 5-pass in-place predicated shift cascade
(binary decomposition of the per-row shift t = 12 - s, t in [0, 26]).
Tables are carry-free fp16, built by one LPC-form scan (state = mask*state + p)
that also regenerates the 0 / total pads every tile. Tent MAC in fp16.

Row mapping is p-major: within a tile of 4096 rows, partition p owns rows
[p*32, (p+1)*32), so all HBM transfers are contiguous 128B+ runs per partition.
"""
import sys
sys.path.insert(0, "/opt/trn_rl_repo")
import numpy as np
from concourse import bass, bacc, mybir, tile
from concourse.bass_utils import run_bass_kernel_spmd

F32 = mybir.dt.float32
F16 = mybir.dt.float16
I32 = mybir.dt.int32
OP = mybir.AluOpType
AF = mybir.ActivationFunctionType

P = 128
A = 51
B_TOTAL = 1048576
N_CORES = 8
BC = B_TOTAL // N_CORES
GAMMA = 0.99
G = 64
TILE = P * G
SP = 80           # padded table width per row (fp16): 13 pad0 | 52 table | 15 pad1
TBL = 13          # column of T[0] within a segment
SMIN, SMAX = -14, 12   # si clamp; t = 12 - si in [0, 26]
PACKED = True     # run the 16/8/4/2 cascade passes on int32-viewed fp16 pairs

import os
ABL = set(os.environ.get("KABL", "").split(","))  # timing-ablation switches


def _host_consts():
    j001n = (-0.01 * np.arange(54, dtype=np.float32))[None, :].repeat(P, 0)
    # edge-correction offsets: [0, -0.99] (low bins), [0.99*i - 50, i=50..52]
    c01 = np.array([0.0, -GAMMA], dtype=np.float32)[None, :].repeat(P, 0)
    c3 = np.array([GAMMA * i - 50.0 for i in (50, 51, 52)],
                  dtype=np.float32)[None, :].repeat(P, 0)
    return j001n, c01, c3


def _build_nc(Bc):
    T = Bc // TILE
    FA = G * A
    FS = G * SP

    nc = bacc.Bacc("TRN2", target_bir_lowering=False, debug=False)
    pr = nc.dram_tensor("pdist", [Bc, A], F32, kind="ExternalInput")
    rr = nc.dram_tensor("reward", [Bc], F32, kind="ExternalInput")
    mm = nc.dram_tensor("mask", [Bc], I32, kind="ExternalInput")
    j001n_c = nc.dram_tensor("j001n", [P, 54], F32, kind="ExternalInput")
    c01_c = nc.dram_tensor("c01", [P, 2], F32, kind="ExternalInput")
    c3_c = nc.dram_tensor("c3", [P, 3], F32, kind="ExternalInput")
    mo = nc.dram_tensor("mout", [Bc, A], F32, kind="ExternalOutput")

    prf = pr[:, :].rearrange("b a -> (b a)")
    mof = mo[:, :].rearrange("b a -> (b a)")

    with tile.TileContext(nc) as tc:
        with tc.tile_pool(name="const", bufs=1) as cpool:
            j001n = cpool.tile([P, 54], F32)
            nc.sync.dma_start(out=j001n[:], in_=j001n_c[:, :])
            c01 = cpool.tile([P, 2], F32, tag="c01")
            nc.sync.dma_start(out=c01[:], in_=c01_c[:, :])
            c3 = cpool.tile([P, 3], F32, tag="c3")
            nc.sync.dma_start(out=c3[:], in_=c3_c[:, :])
            biases = []
            for k in range(3):
                bk = cpool.tile([P, 1], F32, tag=f"bias{k}")
                nc.vector.memset(bk[:], float(k))
                biases.append(bk)
            bone = cpool.tile([P, 1], F32, tag="bone")
            nc.vector.memset(bone[:], 1.0)
            zc16 = cpool.tile([P, 1], F16, tag="zc16")
            nc.vector.memset(zc16[:], 0.0)
            # LPC scan reset mask: 1 everywhere, 0 at segment col 0
            d0m = cpool.tile([P, FS], F16, tag="d0m")
            nc.vector.memset(d0m[:], 1.0)
            nc.vector.memset(
                bass.AP(d0m[:].tensor, d0m[:].offset, [d0m[:].ap[0], [SP, G]]), 0.0)
            # staged scan inputs (double-buffered manually; const cols stay 0)
            staged = []
            for pp in range(2):
                sg = cpool.tile([P, FS], F16, tag=f"staged{pp}")
                nc.vector.memset(sg[:], 0.0)
                staged.append(sg)

            with tc.tile_pool(name="sb", bufs=2) as pool:
                for t in range(T):
                    tbase = t * TILE
                    sg = staged[t % 2]

                    pt = pool.tile([P, FA], F32, tag="pt")
                    nc.sync.dma_start(
                        out=pt[:], in_=bass.AP(prf.tensor, tbase * A,
                                               [[G * A, P], [1, G * A]]))
                    rt = pool.tile([P, G], F32, tag="rt")
                    nc.sync.dma_start(
                        out=rt[:], in_=bass.AP(rr[:].tensor, tbase, [[G, P], [1, G]]))
                    mkt = pool.tile([P, G], I32, tag="mkt")
                    nc.sync.dma_start(
                        out=mkt[:], in_=bass.AP(mm[:].tensor, tbase, [[G, P], [1, G]]))

                    def ptv(off, *dims):
                        h = pt[:]
                        return bass.AP(h.tensor, h.offset + off, [h.ap[0]] + list(dims))

                    # stage p into table slots: staged cols 14..64 <- p[0..50]
                    sgh = sg[:]

                    def sgv(off, *dims):
                        return bass.AP(sgh.tensor, sgh.offset + off,
                                       [sgh.ap[0]] + list(dims))
                    nc.scalar.activation(
                        out=sgv(TBL + 1, [SP, G], [1, A]),
                        in_=ptv(0, [A, G], [1, A]), func=AF.Copy)

                    # LPC scan -> carry-free fp16 padded tables (pads included)
                    X = pool.tile([P, FS], F16, tag="X")
                    nc.vector.tensor_tensor_scan(
                        out=X[:], data0=d0m[:], data1=sg[:], initial=0.0,
                        op0=OP.mult, op1=OP.add)
                    Xh = X[:]

                    def xv(off, *dims):
                        return bass.AP(Xh.tensor, Xh.offset + off,
                                       [Xh.ap[0]] + list(dims))

                    # per-row scalars (baseline math, SMIN/SMAX tightened)
                    notmk = pool.tile([P, G], I32, tag="notmk")
                    nc.vector.tensor_scalar(
                        out=notmk[:], in0=mkt[:], scalar1=1, scalar2=None,
                        op0=OP.bitwise_xor)
                    mf = pool.tile([P, G], F32, tag="mf")
                    nc.vector.tensor_copy(out=mf[:], in_=mkt[:])
                    a1 = pool.tile([P, G], F32, tag="a1")
                    nc.vector.tensor_scalar(out=a1[:], in0=rt[:], scalar1=2.5,
                                            scalar2=0.25, op0=OP.mult, op1=OP.add)
                    qt = pool.tile([P, G], F32, tag="qt")
                    nc.vector.tensor_scalar(out=qt[:], in0=rt[:], scalar1=2.5,
                                            scalar2=25.0, op0=OP.mult, op1=OP.add)
                    nc.vector.tensor_scalar(out=qt[:], in0=qt[:], scalar1=0.0,
                                            scalar2=50.0, op0=OP.max, op1=OP.min)
                    nc.vector.tensor_scalar(out=qt[:], in0=qt[:],
                                            scalar1=GAMMA * 25.0, scalar2=None,
                                            op0=OP.subtract)
                    al = pool.tile([P, G], F32, tag="al")
                    nc.vector.tensor_tensor(out=al[:], in0=a1[:], in1=qt[:],
                                            op=OP.subtract)
                    nc.vector.tensor_tensor(out=al[:], in0=al[:], in1=mf[:],
                                            op=OP.mult)
                    nc.vector.tensor_tensor(out=al[:], in0=al[:], in1=qt[:],
                                            op=OP.add)
                    sf = pool.tile([P, G], F32, tag="sf")
                    nc.vector.tensor_scalar(out=sf[:], in0=al[:], scalar1=-0.5,
                                            scalar2=None, op0=OP.add)
                    si = pool.tile([P, G], I32, tag="si")
                    nc.vector.tensor_copy(out=si[:], in_=sf[:])
                    nc.vector.tensor_scalar(out=si[:], in0=si[:], scalar1=SMIN,
                                            scalar2=SMAX, op0=OP.max, op1=OP.min)
                    nc.vector.tensor_copy(out=sf[:], in_=si[:])
                    rho = pool.tile([P, G], F32, tag="rho")
                    nc.vector.tensor_scalar(out=rho[:], in0=sf[:], scalar1=-GAMMA,
                                            scalar2=-GAMMA, op0=OP.mult, op1=OP.add)
                    nc.vector.tensor_tensor(out=rho[:], in0=rho[:], in1=al[:],
                                            op=OP.add)
                    ti = pool.tile([P, G], I32, tag="ti")
                    nc.vector.tensor_scalar(out=ti[:], in0=si[:], scalar1=-1,
                                            scalar2=12, op0=OP.mult, op1=OP.add)

                    # mask=0 rows: table -> step at virtual atom 25
                    def nmk(n):
                        h = notmk[:]
                        return bass.AP(h.tensor, h.offset, [h.ap[0], [1, G], [0, n]])
                    nc.vector.copy_predicated(
                        out=xv(TBL + 1, [SP, G], [1, 25]), mask=nmk(25),
                        data=bass.AP(zc16[:].tensor, zc16[:].offset,
                                     [zc16[:].ap[0], [0, G], [0, 25]]))
                    nc.vector.copy_predicated(
                        out=xv(TBL + 26, [SP, G], [1, 26]), mask=nmk(26),
                        data=xv(TBL + 51, [SP, G], [0, 26]))

                    # shift cascade: X[seg, i] <- X[seg, i + t] via bits of t
                    bitm = pool.tile([P, G], I32, tag="bitm")
                    if PACKED:
                        X32 = Xh.bitcast(I32)

                        def x32v(off, *dims):
                            return bass.AP(X32.tensor, X32.offset + off,
                                           [X32.ap[0]] + list(dims))
                        for b, w in ((16, 64), (8, 61), (4, 57), (2, 55)):
                            nc.vector.tensor_scalar(
                                out=bitm[:], in0=ti[:], scalar1=b, scalar2=None,
                                op0=OP.bitwise_and)
                            wp = (w + 1) // 2
                            nc.vector.copy_predicated(
                                out=x32v(0, [SP // 2, G], [1, wp]),
                                mask=bass.AP(bitm[:].tensor, bitm[:].offset,
                                             [bitm[:].ap[0], [1, G], [0, wp]]),
                                data=x32v(b // 2, [SP // 2, G], [1, wp]))
                    else:
                        for b, w in ((16, 64), (8, 61), (4, 57), (2, 55)):
                            nc.vector.tensor_scalar(
                                out=bitm[:], in0=ti[:], scalar1=b, scalar2=None,
                                op0=OP.bitwise_and)
                            nc.vector.copy_predicated(
                                out=xv(0, [SP, G], [1, w]),
                                mask=bass.AP(bitm[:].tensor, bitm[:].offset,
                                             [bitm[:].ap[0], [1, G], [0, w]]),
                                data=xv(b, [SP, G], [1, w]))
                    nc.vector.tensor_scalar(
                        out=bitm[:], in0=ti[:], scalar1=1, scalar2=None,
                        op0=OP.bitwise_and)
                    nc.vector.copy_predicated(
                        out=xv(0, [SP, G], [1, 54]),
                        mask=bass.AP(bitm[:].tensor, bitm[:].offset,
                                     [bitm[:].ap[0], [1, G], [0, 54]]),
                        data=xv(1, [SP, G], [1, 54]))

                    # window diffs wd[i] = W[i+1] - W[i], fp16
                    wd = pool.tile([P, G * 54], F16, tag="wd")
                    wdh = wd[:]

                    def wdv(off, *dims):
                        return bass.AP(wdh.tensor, wdh.offset + off,
                                       [wdh.ap[0]] + list(dims))
                    nc.vector.tensor_tensor(
                        out=wdv(0, [54, G], [1, 53]), in0=xv(1, [SP, G], [1, 53]),
                        in1=xv(0, [SP, G], [1, 53]), op=OP.subtract)
                    # aligned copy of wd shifted by 1 (for the k=1 tap), on ACT
                    wdo = pool.tile([P, G * 52], F16, tag="wdo")
                    wdov = bass.AP(wdo[:].tensor, wdo[:].offset,
                                   [wdo[:].ap[0], [52, G], [1, A]])
                    nc.scalar.activation(out=wdov, in_=wdv(1, [54, G], [1, A]),
                                         func=AF.Copy)

                    # tent args Y = rho - 0.01*i
                    Y = pool.tile([P, G * 54], F16, tag="Y")
                    nc.vector.tensor_tensor(
                        out=Y[:],
                        in0=bass.AP(rho[:].tensor, rho[:].offset,
                                    [rho[:].ap[0], [1, G], [0, 54]]),
                        in1=bass.AP(j001n[:].tensor, j001n[:].offset,
                                    [j001n[:].ap[0], [0, G], [1, 54]]),
                        op=OP.add)
                    Yh = Y[:]

                    def yv(off, *dims):
                        return bass.AP(Yh.tensor, Yh.offset + off,
                                       [Yh.ap[0]] + list(dims))

                    # 3-tap tent MAC in fp16 (per-tap tiles so ACT runs ahead)
                    mt_ = pool.tile([P, FA], F16, tag="mt_")
                    au0 = pool.tile([P, FA], F16, tag="au0")
                    au1 = pool.tile([P, FA], F16, tag="au1")
                    tmp1 = pool.tile([P, FA], F16, tag="tmp1")
                    aus = [au0, au1, au0]
                    tmps = [None, tmp1, tmp1]
                    for k in range(3):
                        nc.scalar.activation(
                            out=aus[k][:], in_=yv(k, [54, G], [1, A]),
                            func=AF.Abs, bias=biases[k][:], scale=1.0)
                        nc.scalar.activation(
                            out=aus[k][:], in_=aus[k][:], func=AF.Relu,
                            bias=bone[:], scale=-1.0)
                        wdk = (wdv(0, [54, G], [1, A]) if k == 0 else
                               (bass.AP(wdo[:].tensor, wdo[:].offset,
                                        [wdo[:].ap[0], [52, G], [1, A]])
                                if k == 1 else wdv(2, [54, G], [1, A])))
                        if k == 0:
                            nc.vector.tensor_tensor(
                                out=mt_[:], in0=aus[k][:], in1=wdk, op=OP.mult)
                        else:
                            nc.vector.tensor_tensor(
                                out=tmps[k][:], in0=aus[k][:], in1=wdk, op=OP.mult)
                            nc.vector.tensor_tensor(
                                out=mt_[:], in0=mt_[:], in1=tmps[k][:], op=OP.add)

                    # edge corrections: d0 (bin 0), d5 (bin 50), batched over i
                    d0 = pool.tile([P, G], F32, tag="d0")
                    nc.vector.tensor_copy(out=d0[:], in_=xv(0, [SP, G]))
                    cx01 = pool.tile([P, G * 2], F32, tag="cx01")
                    cx01v = bass.AP(cx01[:].tensor, cx01[:].offset,
                                    [cx01[:].ap[0], [2, G], [1, 2]])
                    nc.vector.tensor_tensor(
                        out=cx01v,
                        in0=bass.AP(c01[:].tensor, c01[:].offset,
                                    [c01[:].ap[0], [0, G], [1, 2]]),
                        in1=bass.AP(rho[:].tensor, rho[:].offset,
                                    [rho[:].ap[0], [1, G], [0, 2]]),
                        op=OP.subtract)
                    nc.vector.tensor_scalar(out=cx01[:], in0=cx01[:], scalar1=0.0,
                                            scalar2=1.0, op0=OP.max, op1=OP.min)
                    nc.vector.tensor_tensor(out=cx01v, in0=cx01v,
                                            in1=wdv(0, [54, G], [1, 2]), op=OP.mult)
                    dc = pool.tile([P, G], F32, tag="dc")
                    nc.vector.tensor_reduce(
                        out=dc[:], in_=bass.AP(cx01[:].tensor, cx01[:].offset,
                                               [cx01[:].ap[0], [2, G], [1, 2]]),
                        axis=mybir.AxisListType.X, op=OP.add)
                    nc.vector.tensor_tensor(out=d0[:], in0=d0[:], in1=dc[:],
                                            op=OP.add)
                    d5 = pool.tile([P, G], F32, tag="d5")
                    nc.vector.tensor_scalar(out=d5[:], in0=xv(53, [SP, G]),
                                            scalar1=-1.0, scalar2=1.0,
                                            op0=OP.mult, op1=OP.add)
                    cx3 = pool.tile([P, G * 3], F32, tag="cx3")
                    cx3v = bass.AP(cx3[:].tensor, cx3[:].offset,
                                   [cx3[:].ap[0], [3, G], [1, 3]])
                    nc.vector.tensor_tensor(
                        out=cx3v,
                        in0=bass.AP(c3[:].tensor, c3[:].offset,
                                    [c3[:].ap[0], [0, G], [1, 3]]),
                        in1=bass.AP(rho[:].tensor, rho[:].offset,
                                    [rho[:].ap[0], [1, G], [0, 3]]),
                        op=OP.add)
                    nc.vector.tensor_scalar(out=cx3[:], in0=cx3[:], scalar1=0.0,
                                            scalar2=1.0, op0=OP.max, op1=OP.min)
                    nc.vector.tensor_tensor(out=cx3v, in0=cx3v,
                                            in1=wdv(50, [54, G], [1, 3]), op=OP.mult)
                    nc.vector.tensor_reduce(
                        out=dc[:], in_=bass.AP(cx3[:].tensor, cx3[:].offset,
                                               [cx3[:].ap[0], [3, G], [1, 3]]),
                        axis=mybir.AxisListType.X, op=OP.add)
                    nc.vector.tensor_tensor(out=d5[:], in0=d5[:], in1=dc[:],
                                            op=OP.add)
                    mh = mt_[:]
                    nc.vector.tensor_tensor(
                        out=bass.AP(mh.tensor, mh.offset, [mh.ap[0], [A, G]]),
                        in0=bass.AP(mh.tensor, mh.offset, [mh.ap[0], [A, G]]),
                        in1=d0[:], op=OP.add)
                    nc.vector.tensor_tensor(
                        out=bass.AP(mh.tensor, mh.offset + 50, [mh.ap[0], [A, G]]),
                        in0=bass.AP(mh.tensor, mh.offset + 50, [mh.ap[0], [A, G]]),
                        in1=d5[:], op=OP.add)

                    # reuse pt's buffer (dead after the stage copy) for the output
                    nc.scalar.activation(out=pt[:], in_=mt_[:], func=AF.Copy)
                    nc.sync.dma_start(
                        out=bass.AP(mof.tensor, tbase * A, [[G * A, P], [1, G * A]]),
                        in_=pt[:])
    nc.compile()
    return nc


_NC_CACHE = {}


def kernel(batch_reward, max_next_dist, supports, non_final_mask):
    assert max_next_dist.shape == (B_TOTAL, A)
    if "nc" not in _NC_CACHE:
        _NC_CACHE["nc"] = _build_nc(BC)
    nc = _NC_CACHE["nc"]
    j001n, c01, c3 = _host_consts()
    in_maps = []
    for c in range(N_CORES):
        s = slice(c * BC, (c + 1) * BC)
        in_maps.append({
            "pdist": np.ascontiguousarray(max_next_dist[s]).astype(np.float32),
            "reward": np.ascontiguousarray(batch_reward[s]).astype(np.float32),
            "mask": np.ascontiguousarray(non_final_mask[s]).astype(np.int32),
            "j001n": j001n, "c01": c01, "c3": c3,
        })
    res = run_bass_kernel_spmd(nc, in_maps, core_ids=list(range(N_CORES)))
    return np.concatenate([res.results[c]["mout"] for c in range(N_CORES)], axis=0)


# revision 22
# speedup vs baseline: 134.8524x; 132.0828x over previous
"""C51 categorical-DQN histogram projection on Trainium2, 8-core data-parallel.

v2: no DRAM round-trip, no indirect DMA. Per-row window extraction from the
in-SBUF padded prefix table via a 5-pass in-place predicated shift cascade
(binary decomposition of the per-row shift t = 12 - s, t in [0, 26]).
Tables are carry-free fp16, built by one LPC-form scan (state = mask*state + p)
that also regenerates the 0 / total pads every tile. Tent MAC in fp16.

Row mapping is p-major: within a tile of 4096 rows, partition p owns rows
[p*32, (p+1)*32), so all HBM transfers are contiguous 128B+ runs per partition.
"""
import sys
sys.path.insert(0, "/opt/trn_rl_repo")
import numpy as np
from concourse import bass, bacc, mybir, tile
from concourse.bass_utils import run_bass_kernel_spmd

F32 = mybir.dt.float32
F16 = mybir.dt.float16
I32 = mybir.dt.int32
OP = mybir.AluOpType
AF = mybir.ActivationFunctionType

P = 128
A = 51
B_TOTAL = 1048576
N_CORES = 8
BC = B_TOTAL // N_CORES
GAMMA = 0.99
G = 64
TILE = P * G
SP = 80           # padded table width per row (fp16): 13 pad0 | 52 table | 15 pad1
TBL = 13          # column of T[0] within a segment
SMIN, SMAX = -14, 12   # si clamp; t = 12 - si in [0, 26]
PACKED = True     # run the 16/8/4/2 cascade passes on int32-viewed fp16 pairs

import os
ABL = set(os.environ.get("KABL", "").split(","))  # timing-ablation switches


def _host_consts():
    j001n = (-0.01 * np.arange(54, dtype=np.float32))[None, :].repeat(P, 0)
    # edge-correction offsets: [0, -0.99] (low bins), [0.99*i - 50, i=50..52]
    c01 = np.array([0.0, -GAMMA], dtype=np.float32)[None, :].repeat(P, 0)
    c3 = np.array([GAMMA * i - 50.0 for i in (50, 51, 52)],
                  dtype=np.float32)[None, :].repeat(P, 0)
    return j001n, c01, c3


def _build_nc(Bc):
    T = Bc // TILE
    FA = G * A
    FS = G * SP

    nc = bacc.Bacc("TRN2", target_bir_lowering=False, debug=False)
    pr = nc.dram_tensor("pdist", [Bc, A], F32, kind="ExternalInput")
    rr = nc.dram_tensor("reward", [Bc], F32, kind="ExternalInput")
    mm = nc.dram_tensor("mask", [Bc], I32, kind="ExternalInput")
    j001n_c = nc.dram_tensor("j001n", [P, 54], F32, kind="ExternalInput")
    c01_c = nc.dram_tensor("c01", [P, 2], F32, kind="ExternalInput")
    c3_c = nc.dram_tensor("c3", [P, 3], F32, kind="ExternalInput")
    mo = nc.dram_tensor("mout", [Bc, A], F32, kind="ExternalOutput")

    prf = pr[:, :].rearrange("b a -> (b a)")
    mof = mo[:, :].rearrange("b a -> (b a)")

    with tile.TileContext(nc) as tc:
        with tc.tile_pool(name="const", bufs=1) as cpool:
            j001n = cpool.tile([P, 54], F32)
            nc.sync.dma_start(out=j001n[:], in_=j001n_c[:, :])
            c01 = cpool.tile([P, 2], F32, tag="c01")
            nc.sync.dma_start(out=c01[:], in_=c01_c[:, :])
            c3 = cpool.tile([P, 3], F32, tag="c3")
            nc.sync.dma_start(out=c3[:], in_=c3_c[:, :])
            biases = []
            for k in range(3):
                bk = cpool.tile([P, 1], F32, tag=f"bias{k}")
                nc.vector.memset(bk[:], float(k))
                biases.append(bk)
            bone = cpool.tile([P, 1], F32, tag="bone")
            nc.vector.memset(bone[:], 1.0)
            zc16 = cpool.tile([P, 1], F16, tag="zc16")
            nc.vector.memset(zc16[:], 0.0)
            zc32 = cpool.tile([P, 1], I32, tag="zc32")
            nc.vector.memset(zc32[:], 0)
            # LPC scan reset mask: 1 everywhere, 0 at segment col 0
            d0m = cpool.tile([P, FS], F16, tag="d0m")
            nc.vector.memset(d0m[:], 1.0)
            nc.vector.memset(
                bass.AP(d0m[:].tensor, d0m[:].offset, [d0m[:].ap[0], [SP, G]]), 0.0)
            # staged scan inputs (double-buffered manually; const cols stay 0)
            staged = []
            for pp in range(2):
                sg = cpool.tile([P, FS], F16, tag=f"staged{pp}")
                nc.vector.memset(sg[:], 0.0)
                staged.append(sg)

            with tc.tile_pool(name="sb", bufs=2) as pool:
                for t in range(T):
                    tbase = t * TILE
                    sg = staged[t % 2]

                    pt = pool.tile([P, FA], F32, tag="pt")
                    nc.sync.dma_start(
                        out=pt[:], in_=bass.AP(prf.tensor, tbase * A,
                                               [[G * A, P], [1, G * A]]))
                    rt = pool.tile([P, G], F32, tag="rt")
                    nc.sync.dma_start(
                        out=rt[:], in_=bass.AP(rr[:].tensor, tbase, [[G, P], [1, G]]))
                    mkt = pool.tile([P, G], I32, tag="mkt")
                    nc.sync.dma_start(
                        out=mkt[:], in_=bass.AP(mm[:].tensor, tbase, [[G, P], [1, G]]))

                    def ptv(off, *dims):
                        h = pt[:]
                        return bass.AP(h.tensor, h.offset + off, [h.ap[0]] + list(dims))

                    # stage p into table slots: staged cols 14..64 <- p[0..50]
                    sgh = sg[:]

                    def sgv(off, *dims):
                        return bass.AP(sgh.tensor, sgh.offset + off,
                                       [sgh.ap[0]] + list(dims))
                    nc.scalar.activation(
                        out=sgv(TBL + 1, [SP, G], [1, A]),
                        in_=ptv(0, [A, G], [1, A]), func=AF.Copy)

                    # LPC scan -> carry-free fp16 padded tables (pads included)
                    X = pool.tile([P, FS], F16, tag="X")
                    nc.vector.tensor_tensor_scan(
                        out=X[:], data0=d0m[:], data1=sg[:], initial=0.0,
                        op0=OP.mult, op1=OP.add)
                    Xh = X[:]

                    def xv(off, *dims):
                        return bass.AP(Xh.tensor, Xh.offset + off,
                                       [Xh.ap[0]] + list(dims))

                    # per-row scalars (baseline math, SMIN/SMAX tightened)
                    notmk = pool.tile([P, G], I32, tag="notmk")
                    nc.vector.tensor_scalar(
                        out=notmk[:], in0=mkt[:], scalar1=1, scalar2=None,
                        op0=OP.bitwise_xor)
                    mf = pool.tile([P, G], F32, tag="mf")
                    nc.vector.tensor_copy(out=mf[:], in_=mkt[:])
                    a1 = pool.tile([P, G], F32, tag="a1")
                    nc.vector.tensor_scalar(out=a1[:], in0=rt[:], scalar1=2.5,
                                            scalar2=0.25, op0=OP.mult, op1=OP.add)
                    qt = pool.tile([P, G], F32, tag="qt")
                    nc.vector.tensor_scalar(out=qt[:], in0=rt[:], scalar1=2.5,
                                            scalar2=25.0, op0=OP.mult, op1=OP.add)
                    nc.vector.tensor_scalar(out=qt[:], in0=qt[:], scalar1=0.0,
                                            scalar2=50.0, op0=OP.max, op1=OP.min)
                    nc.vector.tensor_scalar(out=qt[:], in0=qt[:],
                                            scalar1=GAMMA * 25.0, scalar2=None,
                                            op0=OP.subtract)
                    al = pool.tile([P, G], F32, tag="al")
                    nc.vector.tensor_tensor(out=al[:], in0=a1[:], in1=qt[:],
                                            op=OP.subtract)
                    nc.vector.tensor_tensor(out=al[:], in0=al[:], in1=mf[:],
                                            op=OP.mult)
                    nc.vector.tensor_tensor(out=al[:], in0=al[:], in1=qt[:],
                                            op=OP.add)
                    sf = pool.tile([P, G], F32, tag="sf")
                    nc.vector.tensor_scalar(out=sf[:], in0=al[:], scalar1=-0.5,
                                            scalar2=None, op0=OP.add)
                    si = pool.tile([P, G], I32, tag="si")
                    nc.vector.tensor_copy(out=si[:], in_=sf[:])
                    nc.vector.tensor_scalar(out=si[:], in0=si[:], scalar1=SMIN,
                                            scalar2=SMAX, op0=OP.max, op1=OP.min)
                    nc.vector.tensor_copy(out=sf[:], in_=si[:])
                    rho = pool.tile([P, G], F32, tag="rho")
                    nc.vector.tensor_scalar(out=rho[:], in0=sf[:], scalar1=-GAMMA,
                                            scalar2=-GAMMA, op0=OP.mult, op1=OP.add)
                    nc.vector.tensor_tensor(out=rho[:], in0=rho[:], in1=al[:],
                                            op=OP.add)
                    ti = pool.tile([P, G], I32, tag="ti")
                    nc.vector.tensor_scalar(out=ti[:], in0=si[:], scalar1=-1,
                                            scalar2=12, op0=OP.mult, op1=OP.add)

                    # mask=0 rows: table -> step at virtual atom 25.
                    # fp16 cols 14..37 (T[1..24]) and 40..63 (T[27..50]) are
                    # edited on the packed int32 view; boundary cols 38,39,64
                    # (T[25], T[26], T[51]) individually in fp16.
                    def nmk(n):
                        h = notmk[:]
                        return bass.AP(h.tensor, h.offset, [h.ap[0], [1, G], [0, n]])
                    X32p = Xh.bitcast(I32)

                    def x32pv(off, *dims):
                        return bass.AP(X32p.tensor, X32p.offset + off,
                                       [X32p.ap[0]] + list(dims))
                    nc.vector.copy_predicated(
                        out=x32pv(7, [SP // 2, G], [1, 12]), mask=nmk(12),
                        data=bass.AP(zc32[:].tensor, zc32[:].offset,
                                     [zc32[:].ap[0], [0, G], [0, 12]]))
                    # int32 col 32 = fp16 cols (64, 65) = (T[51], pad) = totals
                    nc.vector.copy_predicated(
                        out=x32pv(20, [SP // 2, G], [1, 12]), mask=nmk(12),
                        data=x32pv(32, [SP // 2, G], [0, 12]))
                    nc.vector.copy_predicated(
                        out=xv(TBL + 25, [SP, G], [1, 1]), mask=nmk(1),
                        data=bass.AP(zc16[:].tensor, zc16[:].offset,
                                     [zc16[:].ap[0], [0, G], [0, 1]]))
                    nc.vector.copy_predicated(
                        out=xv(TBL + 26, [SP, G], [1, 1]), mask=nmk(1),
                        data=xv(TBL + 51, [SP, G], [0, 1]))

                    # shift cascade: X[seg, i] <- X[seg, i + t] via bits of t
                    bitm = pool.tile([P, G], I32, tag="bitm")
                    if PACKED:
                        X32 = Xh.bitcast(I32)

                        def x32v(off, *dims):
                            return bass.AP(X32.tensor, X32.offset + off,
                                           [X32.ap[0]] + list(dims))
                        for b, w in ((16, 64), (8, 61), (4, 57), (2, 55)):
                            nc.vector.tensor_scalar(
                                out=bitm[:], in0=ti[:], scalar1=b, scalar2=None,
                                op0=OP.bitwise_and)
                            wp = (w + 1) // 2
                            nc.vector.copy_predicated(
                                out=x32v(0, [SP // 2, G], [1, wp]),
                                mask=bass.AP(bitm[:].tensor, bitm[:].offset,
                                             [bitm[:].ap[0], [1, G], [0, wp]]),
                                data=x32v(b // 2, [SP // 2, G], [1, wp]))
                    else:
                        for b, w in ((16, 64), (8, 61), (4, 57), (2, 55)):
                            nc.vector.tensor_scalar(
                                out=bitm[:], in0=ti[:], scalar1=b, scalar2=None,
                                op0=OP.bitwise_and)
                            nc.vector.copy_predicated(
                                out=xv(0, [SP, G], [1, w]),
                                mask=bass.AP(bitm[:].tensor, bitm[:].offset,
                                             [bitm[:].ap[0], [1, G], [0, w]]),
                                data=xv(b, [SP, G], [1, w]))
                    nc.vector.tensor_scalar(
                        out=bitm[:], in0=ti[:], scalar1=1, scalar2=None,
                        op0=OP.bitwise_and)
                    nc.vector.copy_predicated(
                        out=xv(0, [SP, G], [1, 54]),
                        mask=bass.AP(bitm[:].tensor, bitm[:].offset,
                                     [bitm[:].ap[0], [1, G], [0, 54]]),
                        data=xv(1, [SP, G], [1, 54]))

                    # window diffs wd[i] = W[i+1] - W[i], fp16
                    wd = pool.tile([P, G * 54], F16, tag="wd")
                    wdh = wd[:]

                    def wdv(off, *dims):
                        return bass.AP(wdh.tensor, wdh.offset + off,
                                       [wdh.ap[0]] + list(dims))
                    nc.vector.tensor_tensor(
                        out=wdv(0, [54, G], [1, 53]), in0=xv(1, [SP, G], [1, 53]),
                        in1=xv(0, [SP, G], [1, 53]), op=OP.subtract)
                    # aligned copy of wd shifted by 1 (for the k=1 tap), on ACT
                    wdo = pool.tile([P, G * 52], F16, tag="wdo")
                    wdov = bass.AP(wdo[:].tensor, wdo[:].offset,
                                   [wdo[:].ap[0], [52, G], [1, A]])
                    nc.scalar.activation(out=wdov, in_=wdv(1, [54, G], [1, A]),
                                         func=AF.Copy)

                    # tent args Y = rho - 0.01*i
                    Y = pool.tile([P, G * 54], F16, tag="Y")
                    nc.vector.tensor_tensor(
                        out=Y[:],
                        in0=bass.AP(rho[:].tensor, rho[:].offset,
                                    [rho[:].ap[0], [1, G], [0, 54]]),
                        in1=bass.AP(j001n[:].tensor, j001n[:].offset,
                                    [j001n[:].ap[0], [0, G], [1, 54]]),
                        op=OP.add)
                    Yh = Y[:]

                    def yv(off, *dims):
                        return bass.AP(Yh.tensor, Yh.offset + off,
                                       [Yh.ap[0]] + list(dims))

                    # 3-tap tent MAC in fp16 (per-tap tiles so ACT runs ahead)
                    mt_ = pool.tile([P, FA], F16, tag="mt_")
                    au0 = pool.tile([P, FA], F16, tag="au0")
                    au1 = pool.tile([P, FA], F16, tag="au1")
                    tmp1 = pool.tile([P, FA], F16, tag="tmp1")
                    aus = [au0, au1, au0]
                    tmps = [None, tmp1, tmp1]
                    for k in range(3):
                        nc.scalar.activation(
                            out=aus[k][:], in_=yv(k, [54, G], [1, A]),
                            func=AF.Abs, bias=biases[k][:], scale=1.0)
                        nc.scalar.activation(
                            out=aus[k][:], in_=aus[k][:], func=AF.Relu,
                            bias=bone[:], scale=-1.0)
                        wdk = (wdv(0, [54, G], [1, A]) if k == 0 else
                               (bass.AP(wdo[:].tensor, wdo[:].offset,
                                        [wdo[:].ap[0], [52, G], [1, A]])
                                if k == 1 else wdv(2, [54, G], [1, A])))
                        if k == 0:
                            nc.vector.tensor_tensor(
                                out=mt_[:], in0=aus[k][:], in1=wdk, op=OP.mult)
                        else:
                            nc.vector.tensor_tensor(
                                out=tmps[k][:], in0=aus[k][:], in1=wdk, op=OP.mult)
                            nc.vector.tensor_tensor(
                                out=mt_[:], in0=mt_[:], in1=tmps[k][:], op=OP.add)

                    # edge corrections: d0 (bin 0), d5 (bin 50), batched over i
                    d0 = pool.tile([P, G], F32, tag="d0")
                    nc.vector.tensor_copy(out=d0[:], in_=xv(0, [SP, G]))
                    cx01 = pool.tile([P, G * 2], F32, tag="cx01")
                    cx01v = bass.AP(cx01[:].tensor, cx01[:].offset,
                                    [cx01[:].ap[0], [2, G], [1, 2]])
                    nc.vector.tensor_tensor(
                        out=cx01v,
                        in0=bass.AP(c01[:].tensor, c01[:].offset,
                                    [c01[:].ap[0], [0, G], [1, 2]]),
                        in1=bass.AP(rho[:].tensor, rho[:].offset,
                                    [rho[:].ap[0], [1, G], [0, 2]]),
                        op=OP.subtract)
                    nc.vector.tensor_scalar(out=cx01[:], in0=cx01[:], scalar1=0.0,
                                            scalar2=1.0, op0=OP.max, op1=OP.min)
                    nc.vector.tensor_tensor(out=cx01v, in0=cx01v,
                                            in1=wdv(0, [54, G], [1, 2]), op=OP.mult)
                    dc = pool.tile([P, G], F32, tag="dc")
                    nc.vector.tensor_reduce(
                        out=dc[:], in_=bass.AP(cx01[:].tensor, cx01[:].offset,
                                               [cx01[:].ap[0], [2, G], [1, 2]]),
                        axis=mybir.AxisListType.X, op=OP.add)
                    nc.vector.tensor_tensor(out=d0[:], in0=d0[:], in1=dc[:],
                                            op=OP.add)
                    d5 = pool.tile([P, G], F32, tag="d5")
                    nc.vector.tensor_scalar(out=d5[:], in0=xv(53, [SP, G]),
                                            scalar1=-1.0, scalar2=1.0,
                                            op0=OP.mult, op1=OP.add)
                    cx3 = pool.tile([P, G * 3], F32, tag="cx3")
                    cx3v = bass.AP(cx3[:].tensor, cx3[:].offset,
                                   [cx3[:].ap[0], [3, G], [1, 3]])
                    nc.vector.tensor_tensor(
                        out=cx3v,
                        in0=bass.AP(c3[:].tensor, c3[:].offset,
                                    [c3[:].ap[0], [0, G], [1, 3]]),
                        in1=bass.AP(rho[:].tensor, rho[:].offset,
                                    [rho[:].ap[0], [1, G], [0, 3]]),
                        op=OP.add)
                    nc.vector.tensor_scalar(out=cx3[:], in0=cx3[:], scalar1=0.0,
                                            scalar2=1.0, op0=OP.max, op1=OP.min)
                    nc.vector.tensor_tensor(out=cx3v, in0=cx3v,
                                            in1=wdv(50, [54, G], [1, 3]), op=OP.mult)
                    nc.vector.tensor_reduce(
                        out=dc[:], in_=bass.AP(cx3[:].tensor, cx3[:].offset,
                                               [cx3[:].ap[0], [3, G], [1, 3]]),
                        axis=mybir.AxisListType.X, op=OP.add)
                    nc.vector.tensor_tensor(out=d5[:], in0=d5[:], in1=dc[:],
                                            op=OP.add)
                    mh = mt_[:]
                    nc.vector.tensor_tensor(
                        out=bass.AP(mh.tensor, mh.offset, [mh.ap[0], [A, G]]),
                        in0=bass.AP(mh.tensor, mh.offset, [mh.ap[0], [A, G]]),
                        in1=d0[:], op=OP.add)
                    nc.vector.tensor_tensor(
                        out=bass.AP(mh.tensor, mh.offset + 50, [mh.ap[0], [A, G]]),
                        in0=bass.AP(mh.tensor, mh.offset + 50, [mh.ap[0], [A, G]]),
                        in1=d5[:], op=OP.add)

                    # reuse pt's buffer (dead after the stage copy) for the output
                    nc.scalar.activation(out=pt[:], in_=mt_[:], func=AF.Copy)
                    nc.sync.dma_start(
                        out=bass.AP(mof.tensor, tbase * A, [[G * A, P], [1, G * A]]),
                        in_=pt[:])
    nc.compile()
    return nc


_NC_CACHE = {}


def kernel(batch_reward, max_next_dist, supports, non_final_mask):
    assert max_next_dist.shape == (B_TOTAL, A)
    if "nc" not in _NC_CACHE:
        _NC_CACHE["nc"] = _build_nc(BC)
    nc = _NC_CACHE["nc"]
    j001n, c01, c3 = _host_consts()
    in_maps = []
    for c in range(N_CORES):
        s = slice(c * BC, (c + 1) * BC)
        in_maps.append({
            "pdist": np.ascontiguousarray(max_next_dist[s]).astype(np.float32),
            "reward": np.ascontiguousarray(batch_reward[s]).astype(np.float32),
            "mask": np.ascontiguousarray(non_final_mask[s]).astype(np.int32),
            "j001n": j001n, "c01": c01, "c3": c3,
        })
    res = run_bass_kernel_spmd(nc, in_maps, core_ids=list(range(N_CORES)))
    return np.concatenate([res.results[c]["mout"] for c in range(N_CORES)], axis=0)
